# revision 53
# baseline (speedup 1.0000x reference)
"""Cross-attention Trainium2 kernel (Bass/Tile), SPMD over 8 NeuronCores.

Problem (hardcoded): x[4,4096,1024], context[4,512,768], Wq[1024,1024],
Wk[768,1024], Wv[768,1024], Wo[1024,1024], bo[1024]; 16 heads, dim 64.
    q = x@Wq; k = ctx@Wk; v = ctx@Wv (per-head 64)
    out = softmax(q k^T / 8) v;  y = out@Wo + bo
Sharding: core i -> (batch b = i//2, query half = i%2, 2048 rows), all 16
heads per core. No collectives; host concatenates the 8 output shards.

Device dataflow (matmul operands bf16, PSUM accumulation fp32):
    QT[d,n]   = Wq^T x^T        (lhsT=Wq chunk, rhs=xT chunk)
    KT[d,m]   = Wk^T ctx^T
    V[m,d]    = ctx Wv          (natural; +ones column per head)
    ET[m,n]   = exp(KT_h^T QT_h)        <- already in lhsT layout for PV
    outT[d,n] = V_aug^T ET      (row 64 = softmax denominators)
    rb        = partition_broadcast(recip(denoms))   (Pool engine)
    y[n,c]    = (outT*rb)^T Wo + bo
The softmax max-subtraction is skipped: scores ~ N(0,1), exp is safe.
The 1/8 scale is folded into Wq on the host.

Schedule: setup phase c-outer-accumulates KT/V/QT(0) across 8 PSUM banks
so the PE consumes each weight chunk as its DMA lands; PE is pre-warmed
with dummy matmuls so the p-state ramp burns during the DMA head start.
Phase B interleaves, per head pair: scores(mc 0,1) | Wo(prev ntile) |
scores(mc 2,3) | QT(next ntile) | PV, ordered so the PE never waits on
the ACT exp chain or the DVE/Pool normalize chain.
"""

import numpy as np
import ml_dtypes

import concourse.bass as bass
import concourse.mybir as mybir
import concourse.tile as tile
from concourse import bacc, library_config
from concourse.bass_utils import run_bass_kernel_spmd

F32 = mybir.dt.float32
F32R = mybir.dt.float32r
BF16 = mybir.dt.bfloat16

B, N, C = 4, 4096, 1024
M, CC = 512, 768
H, D = 16, 64
INNER = H * D          # 1024
NPC = N // 2           # 2048 query rows per core
NT = NPC // 512        # 4 n-tiles of 512
NCHUNK_Q = C // 128    # 8 contraction chunks for Q proj
NCHUNK_K = CC // 128   # 6 contraction chunks for K/V proj
NPAIR = H // 2         # 8 head pairs (2 heads stacked per 128 partitions)
NMC = M // 128         # 4 key chunks
VBLK = D + 1           # 65: V columns + ones column per head

TAGS = []  # (label, instruction-id watermark), for trace attribution


def build_nc() -> bass.Bass:
    nc = bacc.Bacc("TRN2", target_bir_lowering=False, debug=False, num_devices=8)

    xT = nc.dram_tensor("xT", [C, NPC], BF16, kind="ExternalInput")
    ctxT = nc.dram_tensor("ctxT", [CC, M], BF16, kind="ExternalInput")
    wq = nc.dram_tensor("wq", [C, INNER], BF16, kind="ExternalInput")
    wk = nc.dram_tensor("wk", [CC, INNER], BF16, kind="ExternalInput")
    wv = nc.dram_tensor("wv", [CC, INNER], BF16, kind="ExternalInput")
    wo = nc.dram_tensor("wo", [INNER, C], BF16, kind="ExternalInput")
    bo = nc.dram_tensor("bo", [1, C], F32R, kind="ExternalInput")
    y = nc.dram_tensor("y", [NPC, C], F32, kind="ExternalOutput")

    TAGS.clear()
    def _t(label):
        TAGS.append((label, nc.next_id()))

    with tile.TileContext(nc) as tc:
        with tc.tile_pool(name="persist", bufs=1) as pp:
            # ---- persistent SBUF ----
            wq_sb = pp.tile([128, NCHUNK_Q * INNER], BF16)
            wo_sb = pp.tile([128, NCHUNK_Q * C], BF16)
            kt_sb = pp.tile([128, NPAIR * M], BF16)
            v_sb = pp.tile([128, NMC * H * VBLK], BF16)
            qt0_t = [pp.tile([128, 512], BF16, name=f"qt0_{j}") for j in range(NPAIR)]
            ones_sb = pp.tile([128, 128], F32R)
            ones_f32 = pp.tile([128, 128], F32)
            bo_sb = pp.tile([128, C], F32)
            bo_row = pp.tile([1, C], F32R)

            # memset can't write f32r; bounce constants through ACT copies
            nc.vector.memset(ones_f32[:], 1.0)
            nc.scalar.copy(out=ones_sb[:], in_=ones_f32[:])
            nc.scalar.copy(  # ones column (col 64) of every (mc, head) block
                out=v_sb[:].rearrange("p (b q) -> p b q", q=VBLK)[:, :, D : D + 1],
                in_=ones_f32[:, 0 : NMC * H].rearrange("p (b q) -> p b q", q=1),
            )

            # ---- phase A: weights + K/V + QT(0), PE chunk-outer ----
            with (
                tc.tile_pool(name="setup", bufs=1) as sp,
                tc.tile_pool(name="spsum", bufs=8, space="PSUM") as sps,
            ):
                wk_sb = sp.tile([128, NCHUNK_K * INNER], BF16)
                wv_sb = sp.tile([128, NCHUNK_K * INNER], BF16)
                ctx_sb = sp.tile([128, NCHUNK_K * M], BF16)
                x0_sb = sp.tile([128, NCHUNK_Q * 512], BF16)

                # PE warm-up during the DMA head start: the p-state ramp
                # (low->mid->full over ~3us) burns on junk matmuls instead
                # of the first real chunks. fp32 operands so only the Pool
                # memset (not the ACT constant bounce) gates the first one.
                warm = sps.tile([128, 128], F32, tag="sps")
                for _ in range(6):
                    nc.tensor.matmul(
                        warm[:], ones_f32[:, 0:128], ones_f32[:, 0:128],
                        start=True, stop=True,
                    )

                # DMA queue order = dependency order of the PE setup work:
                # (wk,ctx) for KT, wv for V, (wq,x0) for QT(0).
                for c in range(NCHUNK_K):
                    nc.sync.dma_start(
                        out=wk_sb[:, c * INNER : (c + 1) * INNER],
                        in_=wk[c * 128 : (c + 1) * 128, :],
                    )
                    nc.sync.dma_start(
                        out=ctx_sb[:, c * M : (c + 1) * M],
                        in_=ctxT[c * 128 : (c + 1) * 128, :],
                    )
                nc.sync.dma_start(out=bo_row[:], in_=bo[:, :])
                for c in range(NCHUNK_K):
                    nc.sync.dma_start(
                        out=wv_sb[:, c * INNER : (c + 1) * INNER],
                        in_=wv[c * 128 : (c + 1) * 128, :],
                    )
                for c in range(NCHUNK_Q):
                    nc.sync.dma_start(
                        out=wq_sb[:, c * INNER : (c + 1) * INNER],
                        in_=wq[c * 128 : (c + 1) * 128, :],
                    )
                    nc.sync.dma_start(
                        out=x0_sb[:, c * 512 : (c + 1) * 512],
                        in_=xT[c * 128 : (c + 1) * 128, 0:512],
                    )

                # KT per head pair [128 (2 heads d), 512 m]; chunk-outer so
                # each (wk,ctx) chunk is consumed as it lands.
                _t("setup:KT")
                kt_ps = [sps.tile([128, M], F32, tag="sps", name=f"ktps{j}") for j in range(NPAIR)]
                for c in range(NCHUNK_K - 1):
                    for j in range(NPAIR):
                        nc.tensor.matmul(
                            kt_ps[j][:],
                            wk_sb[:, c * INNER + j * 128 : c * INNER + (j + 1) * 128],
                            ctx_sb[:, c * M : (c + 1) * M],
                            start=(c == 0),
                            stop=False,
                        )
                # last chunk per pair with its copy right behind, alternating
                # ACT/DVE so the drain doesn't serialize on one engine and
                # stall the next stage's PSUM-bank reuse
                c = NCHUNK_K - 1
                for j in range(NPAIR):
                    nc.tensor.matmul(
                        kt_ps[j][:],
                        wk_sb[:, c * INNER + j * 128 : c * INNER + (j + 1) * 128],
                        ctx_sb[:, c * M : (c + 1) * M],
                        start=False,
                        stop=True,
                    )
                    if j % 2 == 0:
                        nc.scalar.copy(out=kt_sb[:, j * M : (j + 1) * M], in_=kt_ps[j][:])
                    else:
                        with nc.allow_low_precision(reason="bf16 kt"):
                            nc.vector.tensor_copy(kt_sb[:, j * M : (j + 1) * M], kt_ps[j][:])

                # V natural [m, d] with ones col, chunk-outer
                _t("setup:V")
                v_ps = [sps.tile([128, 512], F32, tag="sps", name=f"vps{t}") for t in range(8)]
                for c in range(NCHUNK_K - 1):
                    for t in range(8):
                        mc, hf = t // 2, t % 2
                        nc.tensor.matmul(
                            v_ps[t][:],
                            ctx_sb[:, c * M + mc * 128 : c * M + (mc + 1) * 128],
                            wv_sb[:, c * INNER + hf * 512 : c * INNER + (hf + 1) * 512],
                            start=(c == 0),
                            stop=False,
                        )
                c = NCHUNK_K - 1
                for t in range(8):
                    mc, hf = t // 2, t % 2
                    nc.tensor.matmul(
                        v_ps[t][:],
                        ctx_sb[:, c * M + mc * 128 : c * M + (mc + 1) * 128],
                        wv_sb[:, c * INNER + hf * 512 : c * INNER + (hf + 1) * 512],
                        start=False,
                        stop=True,
                    )
                    base = mc * H * VBLK + hf * 8 * VBLK
                    dst = v_sb[:, base : base + 8 * VBLK].rearrange(
                        "p (h q) -> p h q", q=VBLK
                    )[:, :, 0:D]
                    src = v_ps[t][:].rearrange("p (h q) -> p h q", q=D)
                    if t % 2 == 0:
                        nc.vector.tensor_copy(dst, src)
                    else:
                        nc.scalar.copy(out=dst, in_=src)

                # QT(0), chunk-outer
                _t("setup:bias")
                for cg in range(2):
                    bps = sps.tile([128, 512], F32, tag="sps")
                    nc.tensor.matmul(
                        bps[:],
                        ones_sb[0:1, 0:128],
                        bo_row[0:1, cg * 512 : (cg + 1) * 512],
                        start=True,
                        stop=True,
                    )
                    nc.scalar.copy(out=bo_sb[:, cg * 512 : (cg + 1) * 512], in_=bps[:])
                _t("setup:QT0")
                q_ps = [sps.tile([128, 512], F32, tag="sps", name=f"qps{j}") for j in range(NPAIR)]
                for c in range(NCHUNK_Q - 1):
                    for j in range(NPAIR):
                        nc.tensor.matmul(
                            q_ps[j][:],
                            wq_sb[:, c * INNER + j * 128 : c * INNER + (j + 1) * 128],
                            x0_sb[:, c * 512 : (c + 1) * 512],
                            start=(c == 0),
                            stop=False,
                        )
                c = NCHUNK_Q - 1
                for i, j in enumerate([NPAIR - 1] + list(range(NPAIR - 1))):
                    nc.tensor.matmul(
                        q_ps[j][:],
                        wq_sb[:, c * INNER + j * 128 : c * INNER + (j + 1) * 128],
                        x0_sb[:, c * 512 : (c + 1) * 512],
                        start=False,
                        stop=True,
                    )
                    if i % 2 == 0:
                        nc.scalar.copy(out=qt0_t[j][:], in_=q_ps[j][:])
                    else:
                        with nc.allow_low_precision(reason="bf16 qt"):
                            nc.vector.tensor_copy(qt0_t[j][:], q_ps[j][:])

                # bias broadcast to all partitions via PE outer product

            # ---- phase B: software-pipelined across 512-query tiles ----
            with (
                tc.tile_pool(name="xt", bufs=4) as xp,
                tc.tile_pool(name="qt", bufs=16) as qp,
                tc.tile_pool(name="et", bufs=8) as ep,
                tc.tile_pool(name="ot", bufs=16) as op,
                tc.tile_pool(name="rb", bufs=4) as rbp,
                tc.tile_pool(name="ysb", bufs=3) as yp,
                tc.tile_pool(name="psA", bufs=2, space="PSUM") as ps_a,
                tc.tile_pool(name="psSC", bufs=2, space="PSUM") as ps_sc,
                tc.tile_pool(name="psPV", bufs=2, space="PSUM") as ps_pv,
            ):
                xt_t = {}
                qt_t = {0: {j: qt0_t[j] for j in range(NPAIR)}}
                ot_t = {}

                def emit_x_dma(nt):
                    # one 3D DMA per ntile: each dma_start holds SP.SEQ/HWDGE
                    # ~650ns, so 8 chunked transfers would throttle the queue
                    t = xp.tile([128, NCHUNK_Q * 512], BF16, tag="xt", name=f"xt{nt}")
                    nc.sync.dma_start(
                        out=t[:].rearrange("p (c i) -> p c i", c=NCHUNK_Q),
                        in_=xT[:, nt * 512 : (nt + 1) * 512].rearrange(
                            "(c p) i -> p c i", p=128
                        ),
                    )
                    xt_t[nt] = t

                def emit_qt_pair(nt, j):
                    if j == 0:
                        qt_t[nt] = {}
                    qt_t[nt][j] = qp.tile(
                        [128, 512], BF16, tag="qt", name=f"qt{nt}_{j}"
                    )
                    qt = qt_t[nt][j]
                    xt = xt_t[nt]
                    qps = ps_a.tile([128, 512], F32, tag="psA")
                    for c in range(NCHUNK_Q):
                        nc.tensor.matmul(
                            qps[:],
                            wq_sb[:, c * INNER + j * 128 : c * INNER + (j + 1) * 128],
                            xt[:, c * 512 : (c + 1) * 512],
                            start=(c == 0),
                            stop=(c == NCHUNK_Q - 1),
                        )
                    nc.scalar.copy(out=qt[:], in_=qps[:])

                # Wo groups accumulate ON TOP of a bias row pre-copied into
                # PSUM (DVE, seeded one group ahead so the PE never waits),
                # and y DMAs straight from PSUM: no post-matmul engine op on
                # the critical path.
                GQ = [(nt, g) for nt in range(NT) for g in range(NPAIR)]
                wo_state = {"ci": 0, "si": 0, "seeded": {}}

                def seed_wo():
                    # keep exactly the next-to-consume group pre-seeded
                    si = wo_state["si"]
                    if si >= len(GQ) or si > wo_state["ci"]:
                        return
                    nt, g = GQ[si]
                    cg = g % 2
                    # the last flush groups run after all PV work: borrow the
                    # idle PV banks so seed/drain/matmul fully pipeline
                    if si >= len(GQ) - 6:
                        yps = ps_pv.tile([128, 512], F32, tag="pv", name=f"yps{nt}_{g}")
                    else:
                        yps = ps_a.tile([128, 512], F32, tag="psA", name=f"yps{nt}_{g}")
                    nc.vector.tensor_copy(
                        yps[:], bo_sb[:, cg * 512 : (cg + 1) * 512]
                    )
                    wo_state["seeded"][si] = yps
                    wo_state["si"] = si + 1

                def emit_wo_group():
                    ci = wo_state["ci"]
                    nt, g = GQ[ci]
                    ns, cg = g // 2, g % 2
                    yps = wo_state["seeded"].pop(ci)
                    for j in range(NPAIR):
                        nc.tensor.matmul(
                            yps[:],
                            ot_t[nt][j][:, ns * 128 : (ns + 1) * 128],
                            wo_sb[:, j * C + cg * 512 : j * C + (cg + 1) * 512],
                            start=False,
                            stop=(j == NPAIR - 1),
                            skip_group_check=True,
                        )
                    # drain on ACT (DVE owns the normalize chains; ACT has
                    # slack), then DMA from SBUF. In the flush DVE is idle:
                    # alternate so back-to-back drains overlap.
                    ysb = yp.tile([128, 512], F32, tag="ysb")
                    if ci >= len(GQ) - 11 and ci % 2 == 0:
                        nc.vector.tensor_copy(ysb[:], yps[:])
                    else:
                        nc.scalar.copy(out=ysb[:], in_=yps[:])
                    nc.sync.dma_start(
                        out=y[
                            nt * 512 + ns * 128 : nt * 512 + (ns + 1) * 128,
                            cg * 512 : (cg + 1) * 512,
                        ],
                        in_=ysb[:],
                    )
                    wo_state["ci"] = ci + 1

                def emit_scores_mcp(nt, j, mcp, ets):
                    qt = qt_t[nt][j]
                    for half in range(2):
                        p0, p1 = half * 64, half * 64 + 64
                        # two m-chunks share a 2-bank psum tile so one ACT
                        # exp instruction covers both (fixed-cost amortize)
                        scps = ps_sc.tile([128, 1024], F32, tag="sc")
                        for k in range(2):
                            mc = 2 * mcp + k
                            nc.tensor.matmul(
                                scps[:, k * 512 : (k + 1) * 512],
                                kt_sb[p0:p1, j * M + mc * 128 : j * M + (mc + 1) * 128],
                                qt[p0:p1, :],
                                start=True,
                                stop=True,
                            )
                        et = ep.tile([128, 1024], BF16, tag="et")
                        nc.scalar.activation(
                            et[:], scps[:], mybir.ActivationFunctionType.Exp
                        )
                        ets[half].append(et)

                def emit_pv_half(nt, j, half, ets):
                    h = 2 * j + half
                    pv = ps_pv.tile([65, 512], F32, tag="pv")
                    for mc in range(NMC):
                        vb = mc * H * VBLK + h * VBLK
                        nc.tensor.matmul(
                            pv[:],
                            v_sb[:, vb : vb + VBLK],
                            ets[half][mc // 2][:, (mc % 2) * 512 : (mc % 2 + 1) * 512],
                            start=(mc == 0),
                            stop=(mc == NMC - 1),
                        )
                    # cross-lane reciprocal of the denominator row straight
                    # to partition 0 (DVE handles mismatched in/out bases),
                    # then Pool broadcast to partitions 0-63 (its ucode reads
                    # the tile's partition 0 regardless of the AP base; row 0
                    # rewriting itself with the same value is benign).
                    rb = rbp.tile([64, 512], F32, tag="rb")
                    with nc.allow_low_precision(reason="softmax denom recip"):
                        nc.vector.reciprocal(rb[0:1, :], pv[64:65, :])
                    nc.gpsimd.partition_broadcast(rb[0:64, :], rb[0:1, :])
                    return pv, rb

                def emit_norm(nt, j, pvrb0, pvrb1):
                    dst = ot_t[nt][j]
                    (pv0, rb0), (pv1, rb1) = pvrb0, pvrb1
                    with nc.allow_low_precision(reason="bf16 attn output"):
                        # h1 writes partitions 64-127 directly: DVE supports
                        # a cross-lane output base, no staging DMA needed
                        nc.vector.tensor_mul(dst[64:128, :], pv1[0:64, :], rb1[0:64, :])
                        nc.vector.tensor_mul(dst[0:64, :], pv0[0:64, :], rb0[0:64, :])

                emit_x_dma(1)
                emit_x_dma(2)  # QT(2) runs during ntile 0
                # wo is consumed starting in ntile 1; its DMA queues after
                # everything the setup pipeline is waiting on
                nc.sync.dma_start(
                    out=wo_sb[:].rearrange("p (c i) -> p c i", c=NCHUNK_Q),
                    in_=wo[:].rearrange("(c p) i -> p c i", p=128),
                )

                # One-pair software lookahead: scores+exp for pair idx+1 are
                # emitted during pair idx, so PV always finds its exps
                # cooked. Per step: sc_mcp0(next) | Wo(prev ntile) |
                # sc_mcp1(next) | QT(next ntile) | PV(cur) | normalize(cur).
                pairs = [(nt, j) for nt in range(NT) for j in range(NPAIR)]

                for nt in range(NT):
                    ot_t[nt] = [
                        op.tile([128, 512], BF16, tag="ot", name=f"ot{nt}_{jj}")
                        for jj in range(NPAIR)
                    ]

                def wo_due(nt, j):
                    # Wo shifted one pair so the first group never waits on
                    # the previous ntile's last normalize chain
                    if nt == NT - 1:
                        # hold 2 groups back: they fill the last pair's
                        # normalize-chain latency before the flush
                        return 1 <= j <= NPAIR - 2
                    return nt > 0 and j > 0

                cur_ets = [[], []]
                _t("n0p0:sc")
                emit_scores_mcp(0, 0, 0, cur_ets)
                emit_scores_mcp(0, 0, 1, cur_ets)
                for idx, (nt, j) in enumerate(pairs):
                    if nt == 0 and j == 0:
                        emit_x_dma(3)
                    nxt = pairs[idx + 1] if idx + 1 < len(pairs) else None
                    nxt_ets = None
                    # interleave: next pair's scores around this pair's
                    # Wo/QT so the scps slot rotation never waits on ACT exp
                    if nxt is not None:
                        nxt_ets = [[], []]
                        _t(f"n{nxt[0]}p{nxt[1]}:sc0")
                        emit_scores_mcp(nxt[0], nxt[1], 0, nxt_ets)
                    # something substantial must sit between the two score
                    # mcp groups, else their shared-slot rotation waits on
                    # the ACT exp: Wo when due, else this pair's QT
                    qt_pending = nt + 1 < NT
                    if wo_due(nt, j):
                        _t(f"n{nt}p{j}:wo")
                        emit_wo_group()
                    elif qt_pending:
                        _t(f"n{nt}p{j}:qt")
                        emit_qt_pair(nt + 1, j)
                        qt_pending = False
                    if nxt is not None:
                        _t(f"n{nxt[0]}p{nxt[1]}:sc1")
                        emit_scores_mcp(nxt[0], nxt[1], 1, nxt_ets)
                    if qt_pending:
                        _t(f"n{nt}p{j}:qt")
                        emit_qt_pair(nt + 1, j)
                    if nt > 0:
                        seed_wo()
                    _t(f"n{nt}p{j}:pv")
                    pvrb0 = emit_pv_half(nt, j, 0, cur_ets)
                    pvrb1 = emit_pv_half(nt, j, 1, cur_ets)
                    _t(f"n{nt}p{j}:norm")
                    emit_norm(nt, j, pvrb0, pvrb1)
                    if nt > 0 and j == NPAIR - 1 and nt < NT - 1:
                        emit_wo_group()
                    cur_ets = nxt_ets
                _t("flush")
                while wo_state["ci"] < len(GQ):
                    emit_wo_group()
                    seed_wo()

    nc.compile()
    return nc


_NC_CACHE = None


def kernel(x, context, Wq, Wk, Wv, Wo, bo, _trace=False, _trace_kwargs=None):
    global _NC_CACHE
    if _NC_CACHE is None:
        _NC_CACHE = build_nc()
    nc = _NC_CACHE

    bf = ml_dtypes.bfloat16
    x = np.asarray(x, np.float32)
    context = np.asarray(context, np.float32)
    wq_s = (np.asarray(Wq, np.float32) * np.float32(D**-0.5)).astype(bf)
    wk = np.asarray(Wk, np.float32).astype(bf)
    wv = np.asarray(Wv, np.float32).astype(bf)
    wo = np.asarray(Wo, np.float32).astype(bf)
    bo2 = np.asarray(bo, np.float32).reshape(1, C)

    in_maps = []
    for i in range(8):
        b, hf = i // 2, i % 2
        in_maps.append(
            {
                "xT": np.ascontiguousarray(x[b, hf * NPC : (hf + 1) * NPC, :].T).astype(bf),
                "ctxT": np.ascontiguousarray(context[b].T).astype(bf),
                "wq": wq_s,
                "wk": wk,
                "wv": wv,
                "wo": wo,
                "bo": bo2,
            }
        )

    kw = {}
    if _trace:
        kw = dict(trace=True, trace_kwargs=_trace_kwargs or {})
    res = run_bass_kernel_spmd(nc, in_maps, list(range(8)), **kw)

    out = np.empty((B, N, C), np.float32)
    for i in range(8):
        b, hf = i // 2, i % 2
        out[b, hf * NPC : (hf + 1) * NPC, :] = res.results[i]["y"]
    if _trace:
        return out, res
    return out
